# revision 1
# baseline (speedup 1.0000x reference)
"""Causal attention kernel for 8 TRN2 NeuronCores (Bass/Tile).

Problem: x[4,4096,512], Wq/Wk/Wv[512,64] ->
    softmax(causal(QK^T)/sqrt(64)) @ V  -> [4,4096,64], fp32.

Sharding: 2 cores per batch element (8 = 4 batches x 2). The two cores of a
pair split the KEY dimension (flash-style partial softmax): each core owns 16
of the 32 key tiles (128 keys each), chosen zigzag so causal work is exactly
balanced AND both cores run the identical instruction stream (SPMD), with the
only per-core difference in input data (gathered key rows + mask thresholds).

Because scaled scores are bounded (|s|<=~9 for this data scale), softmax is
computed shift-free: P = exp(s/8); each core returns partial [PV^T; sum(P)]
of shape [65, 4096]; the host combines pairs: out = (PV_a+PV_b)/(l_a+l_b).

Default mode "bf16" (~90 us/core HW, rel err ~5e-3): the host pre-transposes
x to feature-major bf16, so no on-chip transposes are needed; all matmuls run
bf16 at 1 cyc/row with fp32 PSUM accumulation. QT is duplicated across both
64-partition halves and KT pairs are stacked (even tile rows 0-63, odd rows
64-127) so each pair of K=64 score matmuls row-packs into disjoint halves of
the PE array via tile_position (0,0)/(64,0) and runs concurrently. ST pairs
share a 2-bank PSUM tile so exp runs on [128,1024] chunks on ACT; the causal
mask is an additive -1e9 where j < thr[i], with per-row thresholds from input
data (DVE compare, precomputed per diagonal pair). O^T accumulates as
[V|1]^T @ P in PSUM. Projection work is minced into filler units emitted
between softmax pairs so the PE stream stays dense (keeps the HAM clock gate
warm) while ACT paces the exp chain.

Env override ATTN_MM_MODE: "f32r_all" (~143 us, rel err 4e-4, fp32r = PE
fp32_mode=HIGH single-pass) or "f32" (~264 us, rel err 1e-5, full fp32)
select a slower PE-transpose-based pipeline with higher precision.
"""

import os
import sys
import types

sys.path.insert(0, "/opt/trn_rl_repo")

import numpy as np

# ---------------------------------------------------------------- constants
B, N, D, E = 4, 4096, 512, 64
NKT = N // 128            # 32 global key tiles of 128
LKT = NKT // 2            # 16 key tiles per core
NQS = N // 512            # 8 query slices of 512

# Global key-tile ids per side, ordered so that the causal slice-count
# sequence cnt(g) = 8 - g//4 is identical across sides (SPMD requirement).
SIDE_KTS = [
    [0, 2, 4, 6, 8, 10, 12, 14, 17, 19, 21, 23, 25, 27, 29, 31],
    [1, 3, 5, 7, 9, 11, 13, 15, 16, 18, 20, 22, 24, 26, 28, 30],
]
CNT = [8 - g // 4 for g in SIDE_KTS[0]]   # [8,8,7,7,...,1,1] (both sides)
assert CNT == [8 - g // 4 for g in SIDE_KTS[1]]
FIRST = [8 - c for c in CNT]              # first active q-slice per local tile
MASK_VAL = -1e9
SCALE = 0.125             # 1/sqrt(64)

_CACHE = {}


def _install_ntff_shim():
    """Register the axon NTFF profile hook if the image's antenv lacks it."""
    try:
        import antenv  # noqa: F401
    except ImportError:
        return
    if "antenv.axon_hooks" in sys.modules:
        return
    mod = types.ModuleType("antenv.axon_hooks")
    _hook = [None]
    mod.set_axon_ntff_profile_hook = lambda h: _hook.__setitem__(0, h)
    mod.get_axon_ntff_profile_hook = lambda: _hook[0]
    sys.modules["antenv.axon_hooks"] = mod
    try:
        from trn_agent_boot.trn_boot import _ntff_profile_via_ctypes

        hook = _ntff_profile_via_ctypes("/opt/axon/libaxon_pjrt.so")
        if hook is not None:
            mod.set_axon_ntff_profile_hook(hook)
    except Exception:
        pass


def _emit(tc, aps, xt_dt, att_dt):
    import concourse.bass as bass
    from concourse import mybir
    from concourse.masks import make_identity

    nc = tc.nc
    f32 = mybir.dt.float32
    Exp = mybir.ActivationFunctionType.Exp

    from contextlib import ExitStack

    with ExitStack() as ctx:
        consts = ctx.enter_context(tc.tile_pool(name="consts", bufs=1))
        xrow_p = ctx.enter_context(tc.tile_pool(name="xrow", bufs=4))
        xt_p = ctx.enter_context(tc.tile_pool(name="xt", bufs=2))
        tp_ps = ctx.enter_context(tc.tile_pool(name="tp_ps", bufs=2, space="PSUM"))
        kq_ps = ctx.enter_context(tc.tile_pool(name="kq_ps", bufs=2, space="PSUM"))
        st_ps = ctx.enter_context(tc.tile_pool(name="st_ps", bufs=2, space="PSUM"))
        ot_ps = ctx.enter_context(tc.tile_pool(name="ot_ps", bufs=2, space="PSUM"))
        p_pool = ctx.enter_context(tc.tile_pool(name="p", bufs=3))
        msk_p = ctx.enter_context(tc.tile_pool(name="msk", bufs=2))
        osb_p = ctx.enter_context(tc.tile_pool(name="osb", bufs=2))

        ident = consts.tile([128, 128], f32)
        make_identity(nc, ident)

        w_sb = {}
        for name in ("wq", "wk", "wv"):
            t = consts.tile([128, 4, E], f32, tag=name)
            nc.sync.dma_start(out=t, in_=aps[name].rearrange("(a p) e -> p a e", p=128))
            if xt_dt != f32:
                tr = consts.tile([128, 4, E], xt_dt, tag=name + "r")
                nc.vector.tensor_copy(tr, t)
                t = tr
            w_sb[name] = t
        thr_sb = consts.tile([128, LKT], f32)
        nc.sync.dma_start(out=thr_sb, in_=aps["thr"])
        j_sb = consts.tile([128, 512], f32)
        nc.sync.dma_start(out=j_sb, in_=aps["jio"])

        # per-slot causal masks, precomputed once: -1e9 where j < thr[:, l]
        msk_all = consts.tile([128, LKT, 512], f32, tag="mskall")
        for l in range(LKT):
            nc.vector.tensor_scalar(
                out=msk_all[:, l, :], in0=j_sb,
                scalar1=thr_sb[:, l : l + 1], scalar2=MASK_VAL,
                op0=mybir.AluOpType.is_lt, op1=mybir.AluOpType.mult,
            )

        qt_sb = consts.tile([E, N], att_dt)
        kt_sb = consts.tile([E, N // 2], att_dt)
        vp_sb = consts.tile([128, LKT, E + 1], att_dt)
        if att_dt == f32:
            nc.vector.memset(vp_sb[:, :, E : E + 1], 1.0)
        else:
            ones = consts.tile([128, LKT], f32, tag="ones")
            nc.vector.memset(ones, 1.0)
            nc.vector.tensor_copy(vp_sb[:, :, E : E + 1].squeeze(), ones)

        def load_xt_slice(x_ap, sl):
            """DMA 512 rows of x and PE-transpose into [128d, 4dd, 512tok].

            The 4 d-slice transposes of one token tile share one PSUM bank,
            so PSUM->SBUF moves as a single wide copy per token tile.
            """
            xt = xt_p.tile([128, 4, 512], xt_dt, tag="xt")
            for tt in range(4):
                xr = xrow_p.tile([128, D], f32, tag="xr")
                r0 = 512 * sl + 128 * tt
                nc.sync.dma_start(out=xr, in_=x_ap[r0 : r0 + 128, :])
                ps = tp_ps.tile([128, 4, 128], f32, tag="tp")
                for dd in range(4):
                    nc.tensor.transpose(
                        ps[:, dd, :], xr[:, 128 * dd : 128 * (dd + 1)], ident
                    )
                nc.vector.tensor_copy(xt[:, :, 128 * tt : 128 * (tt + 1)], ps)
            return xt

        # ---- K/V projections from gathered key rows
        for sl in range(4):
            xt = load_xt_slice(aps["xk"], sl)
            ps = kq_ps.tile([128, 512], f32, tag="kq")
            for dd in range(4):
                nc.tensor.matmul(
                    ps[:E, :], lhsT=w_sb["wk"][:, dd, :], rhs=xt[:, dd, :],
                    start=(dd == 0), stop=(dd == 3),
                )
            nc.vector.tensor_copy(kt_sb[:, 512 * sl : 512 * (sl + 1)], ps[:E, :])
            for tt in range(4):
                l = 4 * sl + tt
                vps = kq_ps.tile([128, 512], f32, tag="kq")
                for dd in range(4):
                    nc.tensor.matmul(
                        vps[:, :E], lhsT=xt[:, dd, 128 * tt : 128 * (tt + 1)],
                        rhs=w_sb["wv"][:, dd, :],
                        start=(dd == 0), stop=(dd == 3),
                    )
                nc.vector.tensor_copy(vp_sb[:, l, 0:E], vps[:, :E])

        def project_qt(sl):
            xt = load_xt_slice(aps["xq"], sl)
            ps = kq_ps.tile([128, 512], f32, tag="kq")
            for dd in range(4):
                nc.tensor.matmul(
                    ps[:E, :], lhsT=w_sb["wq"][:, dd, :], rhs=xt[:, dd, :],
                    start=(dd == 0), stop=(dd == 3),
                )
            nc.vector.tensor_copy(qt_sb[:, 512 * sl : 512 * (sl + 1)], ps[:E, :])

        def attention_slice(s):
            ot = ot_ps.tile([E + 1, 512], f32, tag="ot")
            contr = [l for l in range(LKT) if FIRST[l] <= s]
            for idx, l in enumerate(contr):
                st = st_ps.tile([128, 512], f32, tag="st")
                nc.tensor.matmul(
                    st,
                    lhsT=kt_sb[:, 128 * l : 128 * (l + 1)],
                    rhs=qt_sb[:, 512 * s : 512 * (s + 1)],
                    start=True, stop=True,
                )
                if FIRST[l] == s:
                    nc.vector.tensor_add(st, st, msk_all[:, l, :])
                p = p_pool.tile([128, 512], att_dt, tag="p")
                nc.scalar.activation(out=p, in_=st, func=Exp, scale=SCALE)
                nc.tensor.matmul(
                    ot,
                    lhsT=vp_sb[:, l, :],
                    rhs=p,
                    start=(idx == 0), stop=(idx == len(contr) - 1),
                    skip_group_check=True,
                )
            osb = osb_p.tile([E + 1, 512], f32, tag="osb")
            nc.vector.tensor_copy(osb, ot)
            nc.sync.dma_start(out=aps["o"][:, 512 * s : 512 * (s + 1)], in_=osb)

        # ---- interleave Q projection with attention so PE work stays dense
        for s in range(NQS):
            project_qt(s)
            attention_slice(s)


def _emit_bf16(tc, aps):
    """bf16 fast path: x arrives pre-transposed (host) in bf16; all matmuls
    bf16 at 1 cyc/row; ST pairs share 2-bank PSUM tiles so exp runs on
    [128,1024] chunks; fp32 PSUM accumulation throughout."""
    from concourse import mybir
    from contextlib import ExitStack

    nc = tc.nc
    f32 = mybir.dt.float32
    bf16 = mybir.dt.bfloat16
    Exp = mybir.ActivationFunctionType.Exp

    with ExitStack() as ctx:
        consts = ctx.enter_context(tc.tile_pool(name="consts", bufs=1))
        xt_p = ctx.enter_context(tc.tile_pool(name="xt", bufs=4))
        kq_ps = ctx.enter_context(tc.tile_pool(name="kq_ps", bufs=2, space="PSUM"))
        st_ps = ctx.enter_context(tc.tile_pool(name="st_ps", bufs=2, space="PSUM"))
        ot_ps = ctx.enter_context(tc.tile_pool(name="ot_ps", bufs=2, space="PSUM"))
        p_pool = ctx.enter_context(tc.tile_pool(name="p", bufs=6))
        osb_p = ctx.enter_context(tc.tile_pool(name="osb", bufs=2))

        def load_xt_slice(xT_ap, sl):
            xt = xt_p.tile([128, 4, 512], bf16, tag="xt")
            src = xT_ap.rearrange("(a p) n -> p a n", p=128)
            nc.sync.dma_start(out=xt, in_=src[:, :, 512 * sl : 512 * (sl + 1)])
            return xt

        # lazy one-ahead prefetch of x tiles: loads are issued one use-site
        # early in the emission sequence (xt pool keeps 4 slots)
        _xt_cache = {}

        def xt_fetch(which, sl):
            key = (which, sl)
            if key not in _xt_cache:
                _xt_cache[key] = load_xt_slice(aps[which], sl)
            return _xt_cache[key]

        def xt_prefetch(which, sl):
            key = (which, sl)
            if key not in _xt_cache:
                _xt_cache[key] = load_xt_slice(aps[which], sl)

        xt_prefetch("xk", 0)

        w_sb = {}
        for name in ("wk", "wv", "wq"):
            t = consts.tile([128, 4, E], bf16, tag=name)
            nc.sync.dma_start(out=t, in_=aps[name].rearrange("(a p) e -> p a e", p=128))
            w_sb[name] = t
        thr_sb = consts.tile([128, LKT], f32)
        nc.sync.dma_start(out=thr_sb, in_=aps["thr"])
        j_sb = consts.tile([128, 512], f32)
        nc.sync.dma_start(out=j_sb, in_=aps["jio"])

        msk_all = consts.tile([128, LKT, 512], f32, tag="mskall")

        def emit_mask_pair(s):
            # masks for the diagonal pair of slice s (local tiles 2s, 2s+1)
            for l in (2 * s, 2 * s + 1):
                nc.vector.tensor_scalar(
                    out=msk_all[:, l, :], in0=j_sb,
                    scalar1=thr_sb[:, l : l + 1], scalar2=MASK_VAL,
                    op0=mybir.AluOpType.is_lt, op1=mybir.AluOpType.mult,
                )

        # QT duplicated across both 64-partition halves; KT pairs stacked
        # (even local tile in rows 0-63, odd in 64-127) so score matmuls
        # can row-pack two K=64 tiles per PE pass.
        qt_sb = consts.tile([128, N], bf16)
        kt_sb = consts.tile([128, LKT // 2, 128], bf16)
        vp_sb = consts.tile([128, LKT, E + 1], bf16)

        def project_k(sl):
            xt = xt_fetch("xk", sl)
            ps = kq_ps.tile([128, 512], f32, tag="kq")
            for dd in range(4):
                nc.tensor.matmul(
                    ps[0:64, :], lhsT=w_sb["wk"][:, dd, :], rhs=xt[:, dd, :],
                    start=(dd == 0), stop=(dd == 3), tile_position=(0, 0),
                )
                nc.tensor.matmul(
                    ps[64:128, :], lhsT=w_sb["wk"][:, dd, :], rhs=xt[:, dd, :],
                    start=(dd == 0), stop=(dd == 3), tile_position=(0, 64),
                )
            for half in range(2):  # pairs 2*sl and 2*sl+1
                j = 2 * sl + half
                nc.vector.tensor_copy(
                    kt_sb[0:64, j, :], ps[0:64, 256 * half : 256 * half + 128]
                )
                nc.vector.tensor_copy(
                    kt_sb[64:128, j, :], ps[64:128, 256 * half + 128 : 256 * half + 256]
                )

        def project_v_tile(sl, tt):
            xt = xt_fetch("xk", sl)
            l = 4 * sl + tt
            vps = kq_ps.tile([128, 512], f32, tag="kq")
            for dd in range(4):
                nc.tensor.matmul(
                    vps[:, :E], lhsT=xt[:, dd, 128 * tt : 128 * (tt + 1)],
                    rhs=w_sb["wv"][:, dd, :],
                    start=(dd == 0), stop=(dd == 3),
                )
            nc.vector.tensor_copy(vp_sb[:, l, 0:E], vps[:, :E])

        def project_qt(sl):
            xt = xt_fetch("xq", sl)
            ps = kq_ps.tile([128, 512], f32, tag="kq")
            for dd in range(4):
                nc.tensor.matmul(
                    ps[0:64, :], lhsT=w_sb["wq"][:, dd, :], rhs=xt[:, dd, :],
                    start=(dd == 0), stop=(dd == 3), tile_position=(0, 0),
                )
                nc.tensor.matmul(
                    ps[64:128, :], lhsT=w_sb["wq"][:, dd, :], rhs=xt[:, dd, :],
                    start=(dd == 0), stop=(dd == 3), tile_position=(0, 64),
                )
            nc.vector.tensor_copy(qt_sb[:, 512 * sl : 512 * (sl + 1)], ps)

        def attention_slice(s, fillers=()):
            fillers = list(fillers)
            ot = ot_ps.tile([E + 1, 512], f32, tag="ot")
            cols = slice(512 * s, 512 * (s + 1))
            # software-pipelined: PV of pair j issues after ST of pair j+1,
            # so PE never stalls in-order behind the exp of pair j. The
            # diagonal (masked) pair goes first: its longer DVE-add + exp
            # chain hides behind the remaining pairs' score matmuls.
            pending = None  # (p_tile, l0, l1, is_first)
            order = [s] + list(range(s))
            for idx, j in enumerate(order):
                l0, l1 = 2 * j, 2 * j + 1
                stp = st_ps.tile([128, 2, 512], f32, tag="st")
                # the two K=64 score matmuls run concurrently in disjoint
                # 64-row groups of the PE array
                nc.tensor.matmul(
                    stp[:, 0, :], lhsT=kt_sb[0:64, j, :],
                    rhs=qt_sb[0:64, cols], start=True, stop=True,
                    tile_position=(0, 0),
                )
                nc.tensor.matmul(
                    stp[:, 1, :], lhsT=kt_sb[64:128, j, :],
                    rhs=qt_sb[64:128, cols], start=True, stop=True,
                    tile_position=(64, 0),
                )
                if j == s:  # the diagonal pair for this slice
                    nc.vector.tensor_add(stp, stp, msk_all[:, 2 * s : 2 * s + 2, :])
                p = p_pool.tile([128, 2, 512], bf16, tag="p")
                nc.scalar.activation(out=p, in_=stp, func=Exp, scale=SCALE)
                if pending is not None:
                    pp, pl0, pl1, pfirst = pending
                    nc.tensor.matmul(
                        ot, lhsT=vp_sb[:, pl0, :], rhs=pp[:, 0, :],
                        start=pfirst, stop=False, skip_group_check=True,
                    )
                    nc.tensor.matmul(
                        ot, lhsT=vp_sb[:, pl1, :], rhs=pp[:, 1, :],
                        start=False, stop=False, skip_group_check=True,
                    )
                pending = (p, l0, l1, idx == 0)
                if fillers:
                    fillers.pop(0)()
            while fillers:
                fillers.pop(0)()
            pp, pl0, pl1, pfirst = pending
            nc.tensor.matmul(
                ot, lhsT=vp_sb[:, pl0, :], rhs=pp[:, 0, :],
                start=pfirst, stop=False, skip_group_check=True,
            )
            nc.tensor.matmul(
                ot, lhsT=vp_sb[:, pl1, :], rhs=pp[:, 1, :],
                start=False, stop=True, skip_group_check=True,
            )
            osb = osb_p.tile([E + 1, 512], f32, tag="osb")
            nc.vector.tensor_copy(osb, ot)
            nc.sync.dma_start(out=aps["o"][:, 512 * s : 512 * (s + 1)], in_=osb)

        # Emission plan: attention slices carry projection "filler" units
        # (V tiles, K-pair projections, late Q projections) pumped between
        # softmax pairs, so the PE stream stays dense while ACT paces the
        # exp chain. All data dependencies are satisfied one slice ahead.
        V = project_v_tile
        K = project_k
        project_k(0)
        project_v_tile(0, 0)
        project_v_tile(0, 1)
        nc.vector.memset(vp_sb[:, :, E : E + 1], 1.0)
        project_qt(0)
        xt_prefetch("xq", 1)
        xt_prefetch("xk", 1)
        emit_mask_pair(0)
        attention_slice(0, [lambda: V(0, 2), lambda: V(0, 3)])
        project_qt(1)
        xt_prefetch("xq", 2)
        emit_mask_pair(1)
        attention_slice(1, [lambda: K(1), lambda: V(1, 0), lambda: V(1, 1)])
        project_qt(2)
        xt_prefetch("xq", 3)
        xt_prefetch("xk", 2)
        emit_mask_pair(2)
        attention_slice(2, [lambda: V(1, 2), lambda: V(1, 3)])
        project_qt(3)
        xt_prefetch("xq", 4)
        emit_mask_pair(3)
        attention_slice(3, [lambda: K(2), lambda: V(2, 0), lambda: V(2, 1)])
        project_qt(4)
        xt_prefetch("xq", 5)
        xt_prefetch("xk", 3)
        emit_mask_pair(4)
        attention_slice(4, [lambda: V(2, 2), lambda: V(2, 3)])
        project_qt(5)
        xt_prefetch("xq", 6)
        emit_mask_pair(5)
        attention_slice(5, [lambda: K(3), lambda: V(3, 0), lambda: V(3, 1)])
        project_qt(6)
        xt_prefetch("xq", 7)
        emit_mask_pair(6)
        attention_slice(6, [lambda: project_qt(7)])
        emit_mask_pair(7)
        attention_slice(7, [lambda: (V(3, 2), V(3, 3))])


def _build(mm_mode):
    import concourse.tile as tile
    from concourse import bacc, mybir

    key = mm_mode
    if key in _CACHE:
        return _CACHE[key]

    f32 = mybir.dt.float32
    f32r = mybir.dt.float32r
    bf16 = mybir.dt.bfloat16

    nc = bacc.Bacc("TRN2", target_bir_lowering=False, debug=False, num_devices=8)
    x_dt = bf16 if mm_mode == "bf16" else f32
    aps = {
        "thr": nc.dram_tensor("thr", [128, LKT], f32, kind="ExternalInput").ap(),
        "jio": nc.dram_tensor("jio", [128, 512], f32, kind="ExternalInput").ap(),
        "o": nc.dram_tensor("o", [E + 1, N], f32, kind="ExternalOutput").ap(),
    }
    for name in ("wq", "wk", "wv"):
        aps[name] = nc.dram_tensor(name, [D, E], x_dt, kind="ExternalInput").ap()
    if mm_mode == "bf16":
        # host supplies x pre-transposed (feature-major) in bf16
        aps["xq"] = nc.dram_tensor("xq", [D, N], bf16, kind="ExternalInput").ap()
        aps["xk"] = nc.dram_tensor("xk", [D, N // 2], bf16, kind="ExternalInput").ap()
        with tile.TileContext(nc) as tc:
            _emit_bf16(tc, aps)
    else:
        xt_dt, att_dt = {
            "f32": (f32, f32),
            "f32r": (f32, f32r),
            "f32r_all": (f32r, f32r),
        }[mm_mode]
        aps["xq"] = nc.dram_tensor("xq", [N, D], f32, kind="ExternalInput").ap()
        aps["xk"] = nc.dram_tensor("xk", [N // 2, D], f32, kind="ExternalInput").ap()
        with tile.TileContext(nc) as tc:
            _emit(tc, aps, xt_dt, att_dt)
    nc.compile()
    _CACHE[key] = nc
    return nc


def make_in_maps(x, Wq, Wk, Wv, mm_mode="f32"):
    x = np.ascontiguousarray(np.asarray(x, dtype=np.float32))
    Wq = np.ascontiguousarray(np.asarray(Wq, dtype=np.float32))
    Wk = np.ascontiguousarray(np.asarray(Wk, dtype=np.float32))
    Wv = np.ascontiguousarray(np.asarray(Wv, dtype=np.float32))
    jio = np.ascontiguousarray(
        np.broadcast_to(np.arange(512, dtype=np.float32), (128, 512))
    )
    bf16_mode = mm_mode == "bf16"
    if bf16_mode:
        import ml_dtypes

        bf = ml_dtypes.bfloat16
        Wq, Wk, Wv = Wq.astype(bf), Wk.astype(bf), Wv.astype(bf)
        xT = [np.ascontiguousarray(x[b].T.astype(bf)) for b in range(B)]
    in_maps = []
    for c in range(8):
        b, side = c // 2, c % 2
        kts = SIDE_KTS[side]
        thr = np.empty((128, LKT), np.float32)
        rows = np.arange(128, dtype=np.float32)
        for l, g in enumerate(kts):
            thr[:, l] = 128 * (g % 4) + rows
        if bf16_mode:
            xq_in = xT[b]
            xk_in = np.ascontiguousarray(
                np.concatenate(
                    [xT[b][:, 128 * g : 128 * (g + 1)] for g in kts], axis=1
                )
            )
        else:
            xq_in = x[b]
            xk_in = np.ascontiguousarray(
                np.concatenate([x[b, 128 * g : 128 * (g + 1)] for g in kts], axis=0)
            )
        in_maps.append(
            {
                "xq": xq_in, "xk": xk_in,
                "wq": Wq, "wk": Wk, "wv": Wv,
                "thr": thr, "jio": jio,
            }
        )
    return in_maps


def combine(results):
    """results: list of 8 dicts with 'o' [65, 4096] -> full output [4,4096,64]."""
    out = np.empty((B, N, E), np.float32)
    for b in range(B):
        oA = results[2 * b]["o"]
        oB = results[2 * b + 1]["o"]
        num = oA[:E] + oB[:E]
        den = oA[E] + oB[E]
        out[b] = (num / den).T
    return out


def _run(inputs, trace=False, tmpdir=None, mm_mode=None):
    from concourse.bass_utils import run_bass_kernel_spmd

    if mm_mode is None:
        mm_mode = os.environ.get("ATTN_MM_MODE", "bf16")
    if trace:
        _install_ntff_shim()
    nc = _build(mm_mode)
    in_maps = make_in_maps(**inputs, mm_mode=mm_mode)
    res = run_bass_kernel_spmd(
        nc, in_maps, core_ids=list(range(8)), trace=trace, tmpdir=tmpdir
    )
    return combine(res.results), res


def kernel(x, Wq, Wk, Wv):
    out, _ = _run({"x": x, "Wq": Wq, "Wk": Wk, "Wv": Wv})
    return out



# revision 10
# speedup vs baseline: 1.2771x; 1.2771x over previous
"""Causal attention kernel for 8 TRN2 NeuronCores (Bass/Tile).

Problem: x[4,4096,512], Wq/Wk/Wv[512,64] ->
    softmax(causal(QK^T)/sqrt(64)) @ V  -> [4,4096,64], fp32.

Sharding: 2 cores per batch element (8 = 4 batches x 2). The two cores of a
pair split the KEY dimension (flash-style partial softmax): each core owns 16
of the 32 key tiles (128 keys each), chosen zigzag so causal work is exactly
balanced AND both cores run the identical instruction stream (SPMD), with the
only per-core difference in input data (gathered key rows + mask thresholds).

Because scaled scores are bounded (|s|<=~9 for this data scale), softmax is
computed shift-free: P = exp(s/8); each core returns partial [PV^T; sum(P)]
of shape [65, 4096]; the host combines pairs: out = (PV_a+PV_b)/(l_a+l_b).

Pipeline structure: the kernel is one flat software pipeline over the 36
(q-slice, key-pair) steps, paced by the ScalarE exp chain (the only
irreducible serial resource at ~1.15us per [128,1024] exp).  Each step emits
[exp_i | ST_{i+1} | filler units | PV_{i-1}] so the PE always has the next
score matmul ready before the current exp retires.  Projections (Q/K with
host-duplicated [128,128] weights = 1 matmul per 128-d chunk; V tiles) are
minced into ~250-550ns units and drained from a deadline-ordered queue in the
slack the PE has under each exp.  The diagonal (masked) pair runs LAST in its
slice so its DVE mask-add never gates a slice transition.
"""

import os
import sys
import types

sys.path.insert(0, "/opt/trn_rl_repo")

import numpy as np

# ---------------------------------------------------------------- constants
B, N, D, E = 4, 4096, 512, 64
NKT = N // 128            # 32 global key tiles of 128
LKT = NKT // 2            # 16 key tiles per core
NQS = N // 512            # 8 query slices of 512

# Global key-tile ids per side, ordered so that the causal slice-count
# sequence cnt(g) = 8 - g//4 is identical across sides (SPMD requirement).
SIDE_KTS = [
    [0, 2, 4, 6, 8, 10, 12, 14, 17, 19, 21, 23, 25, 27, 29, 31],
    [1, 3, 5, 7, 9, 11, 13, 15, 16, 18, 20, 22, 24, 26, 28, 30],
]
CNT = [8 - g // 4 for g in SIDE_KTS[0]]
assert CNT == [8 - g // 4 for g in SIDE_KTS[1]]
MASK_VAL = -1e9
SCALE = 0.125             # 1/sqrt(64)

# PE-filler budget per pipeline step (ns of estimated PE issue time)
FILL_BUDGET = float(os.environ.get("ATTN_FILL_BUDGET", "520"))

_CACHE = {}


def _install_ntff_shim():
    """Register the axon NTFF profile hook if the image's antenv lacks it."""
    try:
        import antenv  # noqa: F401
    except ImportError:
        return
    if "antenv.axon_hooks" in sys.modules:
        return
    mod = types.ModuleType("antenv.axon_hooks")
    _hook = [None]
    mod.set_axon_ntff_profile_hook = lambda h: _hook.__setitem__(0, h)
    mod.get_axon_ntff_profile_hook = lambda: _hook[0]
    sys.modules["antenv.axon_hooks"] = mod
    try:
        from trn_agent_boot.trn_boot import _ntff_profile_via_ctypes

        hook = _ntff_profile_via_ctypes("/opt/axon/libaxon_pjrt.so")
        if hook is not None:
            mod.set_axon_ntff_profile_hook(hook)
    except Exception:
        pass


def _emit_bf16(tc, aps):
    from concourse import mybir
    from contextlib import ExitStack

    nc = tc.nc
    f32 = mybir.dt.float32
    bf16 = mybir.dt.bfloat16
    Exp = mybir.ActivationFunctionType.Exp

    def G(s, j):  # global step index of (slice s, key-pair j)
        return s * (s + 1) // 2 + j

    SEQ = [(s, j) for s in range(NQS) for j in range(s + 1)]  # diag LAST
    NP = len(SEQ)

    with ExitStack() as ctx:
        consts = ctx.enter_context(tc.tile_pool(name="consts", bufs=1))
        xt_p = ctx.enter_context(tc.tile_pool(name="xt", bufs=8))
        kq_ps = ctx.enter_context(tc.tile_pool(name="kq_ps", bufs=2, space="PSUM"))
        st_ps = ctx.enter_context(tc.tile_pool(name="st_ps", bufs=2, space="PSUM"))
        ot_ps = ctx.enter_context(tc.tile_pool(name="ot_ps", bufs=2, space="PSUM"))
        p_pool = ctx.enter_context(tc.tile_pool(name="p", bufs=6))
        osb_p = ctx.enter_context(tc.tile_pool(name="osb", bufs=2))

        wcat = consts.tile([128, 4, 320], bf16)      # [wk|wk | wq|wq | wv]
        tj = consts.tile([128, LKT + 512], f32)      # [thr | jio]
        qt_sb = consts.tile([128, N], bf16)          # Q^T duplicated halves
        kt_sb = consts.tile([128, LKT // 2, 128], bf16)  # stacked pairs
        vp_sb = consts.tile([128, LKT, E + 1], bf16)     # [V | 1]
        msk_all = consts.tile([128, LKT, 512], bf16)     # 0/1 keep-mask

        wk2 = wcat[:, :, 0:128]
        wq2 = wcat[:, :, 128:256]
        wv = wcat[:, :, 256:320]
        thr_sb = tj[:, 0:LKT]
        j_sb = tj[:, LKT : LKT + 512]

        # ------------------------------------------------ DMA / projections
        xt_cache = {}

        def dma_x(which, sl):
            key = (which, sl)
            if key in xt_cache:
                return
            t = xt_p.tile([128, 4, 512], bf16, tag="xt", name="xt")
            src = aps[which].rearrange("(a p) n -> p a n", p=128)
            nc.sync.dma_start(out=t, in_=src[:, :, 512 * sl : 512 * (sl + 1)])
            xt_cache[key] = t

        q_tiles = {}

        def q_mm(s, dd):
            if dd == 0:
                q_tiles[s] = kq_ps.tile([128, 512], f32, tag="kq", name="kq")
            nc.tensor.matmul(
                q_tiles[s], lhsT=wq2[:, dd, :], rhs=xt_cache[("xq", s)][:, dd, :],
                start=(dd == 0), stop=(dd == 3), skip_group_check=True,
            )

        def q_copy(s):
            nc.vector.tensor_copy(qt_sb[:, 512 * s : 512 * (s + 1)], q_tiles.pop(s))

        k_tiles = {}

        def k_mm(sl, dd):
            if dd == 0:
                k_tiles[sl] = kq_ps.tile([128, 512], f32, tag="kq", name="kq")
            nc.tensor.matmul(
                k_tiles[sl], lhsT=wk2[:, dd, :], rhs=xt_cache[("xk", sl)][:, dd, :],
                start=(dd == 0), stop=(dd == 3), skip_group_check=True,
            )

        def k_copy(sl, half):
            ps = k_tiles[sl] if half == 0 else k_tiles.pop(sl)
            j = 2 * sl + half
            c0 = 256 * half
            nc.vector.tensor_copy(kt_sb[0:64, j, :], ps[0:64, c0 : c0 + 128])
            nc.vector.tensor_copy(kt_sb[64:128, j, :], ps[64:128, c0 + 128 : c0 + 256])

        v_tiles = {}

        def v_mm2(sl, tt, part):
            l = 4 * sl + tt
            if part == 0:
                v_tiles[l] = kq_ps.tile([128, 512], f32, tag="kq", name="kq")
            vps = v_tiles[l]
            xt = xt_cache[("xk", sl)]
            for dd in (2 * part, 2 * part + 1):
                nc.tensor.matmul(
                    vps[:, :E], lhsT=xt[:, dd, 128 * tt : 128 * (tt + 1)],
                    rhs=wv[:, dd, :],
                    start=(dd == 0), stop=(dd == 3), skip_group_check=True,
                )
            if part == 1:
                nc.vector.tensor_copy(vp_sb[:, l, 0:E], v_tiles.pop(l)[:, :E])

        def mask2(s):
            # 0/1 keep-mask: 1 where query j >= first allowed (thr), else 0.
            # Applied multiplicatively to P *after* exp (scores are bounded,
            # so the unmasked exp cannot overflow) — keeps the ScalarE exp
            # chain free of any DVE dependency.
            for l in (2 * s, 2 * s + 1):
                nc.vector.tensor_scalar(
                    out=msk_all[:, l, :], in0=j_sb,
                    scalar1=thr_sb[:, l : l + 1], scalar2=1.0,
                    op0=mybir.AluOpType.is_ge, op1=mybir.AluOpType.mult,
                )

        # ------------------------------------------------ attention pieces
        st_tiles, p_tiles, ot_tiles = {}, {}, {}

        def emit_st(i):
            s, j = SEQ[i]
            st = st_ps.tile([128, 2, 512], f32, tag="st", name="st")
            cols = slice(512 * s, 512 * (s + 1))
            nc.tensor.matmul(
                st[:, 0, :], lhsT=kt_sb[0:64, j, :], rhs=qt_sb[0:64, cols],
                start=True, stop=True, tile_position=(0, 0),
            )
            nc.tensor.matmul(
                st[:, 1, :], lhsT=kt_sb[64:128, j, :], rhs=qt_sb[64:128, cols],
                start=True, stop=True, tile_position=(64, 0),
            )
            st_tiles[i] = st

        def emit_exp(i):
            p = p_pool.tile([128, 2, 512], bf16, tag="p", name="p")
            nc.scalar.activation(out=p, in_=st_tiles.pop(i), func=Exp, scale=SCALE)
            p_tiles[i] = p

        def emit_pv(i):
            s, j = SEQ[i]
            if j == 0:
                ot_tiles[s] = ot_ps.tile([E + 1, 512], f32, tag="ot", name="ot")
            ot = ot_tiles[s]
            p = p_tiles.pop(i)
            if j == s:  # diagonal pair: zero the causally-masked P entries
                nc.vector.tensor_mul(p, p, msk_all[:, 2 * s : 2 * s + 2, :])
            for h in (0, 1):
                nc.tensor.matmul(
                    ot, lhsT=vp_sb[:, 2 * j + h, :], rhs=p[:, h, :],
                    start=(j == 0 and h == 0), stop=(j == s and h == 1),
                    skip_group_check=True,
                )

        def emit_out(s):
            osb = osb_p.tile([E + 1, 512], f32, tag="osb", name="osb")
            nc.vector.tensor_copy(osb, ot_tiles.pop(s))
            nc.sync.dma_start(out=aps["o"][:, 512 * s : 512 * (s + 1)], in_=osb)

        # ------------------------------------------------ filler unit queue
        # unit = [before_step, phase, cost_ns, fn]; 'pre' units are deps of
        # ST(before_step) and must emit before it; 'post' units just need to
        # land by then.  Queue is kept in deadline order; pops are FIFO so
        # kq_ps groups never have >2 generations in flight.
        Uq = []

        def add(before, phase, cost, fn):
            Uq.append((before, phase, cost, fn))

        for s in range(1, NQS):
            b = G(s, 0)
            if s >= 2:
                lead = G(s - 2, 0) if s == 2 else G(s - 1, 0)
                add(lead, "pre", 0.0, (lambda s=s: dma_x("xq", s)))
            for dd in range(4):
                add(b, "pre", 240.0, (lambda s=s, dd=dd: q_mm(s, dd)))
            add(b, "pre", 0.0, (lambda s=s: q_copy(s)))
            # mask consumed on DVE just before PV of the diag pair (step G(s,s)+1)
            add(G(s, s) + 1, "post", 0.0, (lambda s=s: mask2(s)))
        for sl in range(1, 4):
            b = G(2 * sl, 2 * sl)
            if sl >= 2:
                add(G(2 * sl - 3, 0), "pre", 0.0, (lambda sl=sl: dma_x("xk", sl)))
            for dd in range(4):
                add(b, "pre", 240.0, (lambda sl=sl, dd=dd: k_mm(sl, dd)))
            for half in (0, 1):
                add(b, "pre", 0.0, (lambda sl=sl, half=half: k_copy(sl, half)))
        for sl in range(4):
            for tt in range(4):
                if sl == 0 and tt < 2:
                    continue  # emitted via early deadline below
                l = 4 * sl + tt
                pz = l // 2
                b = min(G(pz, pz) + 1, NP)
                for part in (0, 1):
                    add(b, "post", 280.0,
                        (lambda sl=sl, tt=tt, part=part: v_mm2(sl, tt, part)))
        # V tiles 0,1 are consumed by PV(0,0) at step 1
        for tt in (0, 1):
            for part in (0, 1):
                add(1, "post", 280.0, (lambda tt=tt, part=part: v_mm2(0, tt, part)))
        Uq.sort(key=lambda u: u[0])  # stable: groups stay contiguous

        def pop_due(i, phase):
            # Emit every due unit of `phase`, in FIFO order, even when a due
            # unit of the other phase sits ahead of it in the queue (the ST
            # about to be emitted depends on its due 'pre' units).
            k = 0
            while k < len(Uq) and Uq[k][0] <= i + 1:
                if Uq[k][1] == phase:
                    Uq.pop(k)[3]()
                else:
                    k += 1

        def pop_budget(budget):
            while Uq and Uq[0][2] <= budget:
                u = Uq.pop(0)
                budget -= u[2]
                u[3]()
            return budget

        # ------------------------------------------------ prologue
        nc.sync.dma_start(out=wcat, in_=aps["wcat"].rearrange("(a p) e -> p a e", p=128))
        dma_x("xk", 0)
        dma_x("xq", 0)
        nc.sync.dma_start(out=tj, in_=aps["tj"])
        dma_x("xq", 1)
        dma_x("xk", 1)
        nc.vector.memset(vp_sb[:, :, E : E + 1], 1.0)
        for dd in range(4):
            k_mm(0, dd)
        k_copy(0, 0)
        k_copy(0, 1)
        for dd in range(4):
            q_mm(0, dd)
        q_copy(0)
        mask2(0)
        emit_st(0)

        # ------------------------------------------------ pipeline
        pending = None
        for i in range(NP):
            emit_exp(i)
            pop_due(i, "pre")
            if i + 1 < NP:
                emit_st(i + 1)
            pop_due(i, "post")
            pop_budget(FILL_BUDGET)
            if pending is not None:
                emit_pv(pending)
                ps, pj = SEQ[pending]
                if ps == pj:  # that pair closed slice ps
                    emit_out(ps)
            pending = i
        while Uq:
            Uq.pop(0)[3]()
        emit_pv(pending)
        emit_out(NQS - 1)


def _build(mm_mode):
    import concourse.tile as tile
    from concourse import bacc, mybir

    key = mm_mode
    if key in _CACHE:
        return _CACHE[key]
    assert mm_mode == "bf16", f"only bf16 mode is supported, got {mm_mode}"

    f32 = mybir.dt.float32
    bf16 = mybir.dt.bfloat16

    nc = bacc.Bacc("TRN2", target_bir_lowering=False, debug=False, num_devices=8)
    aps = {
        "wcat": nc.dram_tensor("wcat", [D, 320], bf16, kind="ExternalInput").ap(),
        "tj": nc.dram_tensor("tj", [128, LKT + 512], f32, kind="ExternalInput").ap(),
        "o": nc.dram_tensor("o", [E + 1, N], f32, kind="ExternalOutput").ap(),
        # host supplies x pre-transposed (feature-major) in bf16
        "xq": nc.dram_tensor("xq", [D, N], bf16, kind="ExternalInput").ap(),
        "xk": nc.dram_tensor("xk", [D, N // 2], bf16, kind="ExternalInput").ap(),
    }
    with tile.TileContext(nc) as tc:
        _emit_bf16(tc, aps)
    nc.compile()
    _CACHE[key] = nc
    return nc


def make_in_maps(x, Wq, Wk, Wv, mm_mode="bf16"):
    import ml_dtypes

    bf = ml_dtypes.bfloat16
    x = np.ascontiguousarray(np.asarray(x, dtype=np.float32))
    Wq = np.asarray(Wq, dtype=np.float32)
    Wk = np.asarray(Wk, dtype=np.float32)
    Wv = np.asarray(Wv, dtype=np.float32)

    wcat = np.empty((D, 320), np.float32)
    wcat[:, 0:64] = Wk
    wcat[:, 64:128] = Wk
    wcat[:, 128:192] = Wq
    wcat[:, 192:256] = Wq
    wcat[:, 256:320] = Wv
    wcat = np.ascontiguousarray(wcat.astype(bf))

    xT = [np.ascontiguousarray(x[b].T.astype(bf)) for b in range(B)]

    jio = np.broadcast_to(np.arange(512, dtype=np.float32), (128, 512))
    in_maps = []
    for c in range(8):
        b, side = c // 2, c % 2
        kts = SIDE_KTS[side]
        tj = np.empty((128, LKT + 512), np.float32)
        rows = np.arange(128, dtype=np.float32)
        for l, g in enumerate(kts):
            tj[:, l] = 128 * (g % 4) + rows
        tj[:, LKT:] = jio
        xk_in = np.ascontiguousarray(
            np.concatenate([xT[b][:, 128 * g : 128 * (g + 1)] for g in kts], axis=1)
        )
        in_maps.append({"xq": xT[b], "xk": xk_in, "wcat": wcat, "tj": tj})
    return in_maps


def combine(results):
    """results: list of 8 dicts with 'o' [65, 4096] -> full output [4,4096,64]."""
    out = np.empty((B, N, E), np.float32)
    for b in range(B):
        oA = results[2 * b]["o"]
        oB = results[2 * b + 1]["o"]
        num = oA[:E] + oB[:E]
        den = oA[E] + oB[E]
        out[b] = (num / den).T
    return out


def _run(inputs, trace=False, tmpdir=None, mm_mode=None):
    from concourse.bass_utils import run_bass_kernel_spmd

    if mm_mode is None:
        mm_mode = os.environ.get("ATTN_MM_MODE", "bf16")
    if trace:
        _install_ntff_shim()
    nc = _build(mm_mode)
    in_maps = make_in_maps(**inputs, mm_mode=mm_mode)
    res = run_bass_kernel_spmd(
        nc, in_maps, core_ids=list(range(8)), trace=trace, tmpdir=tmpdir
    )
    return combine(res.results), res


def kernel(x, Wq, Wk, Wv):
    out, _ = _run({"x": x, "Wq": Wq, "Wk": Wk, "Wv": Wv})
    return out
